# revision 29
# baseline (speedup 1.0000x reference)
"""AGATCellWithMLP Trainium2 kernel: 8-core data-parallel over batch B.

v2 design (one graph per core, everything transposed [channel, node]):
 - Host sends combT pre-transposed bf16 with channels reordered [h | x] so
   that every on-chip partition slice lands on a 0/32/64 base.
 - W1 is folded into Wv host-side (U_h = Wv_h @ W1_h), so the attention
   numerator matmuls directly produce the MLP hidden pre-activations; the
   bv@W1 term is exact via the softmax denominator (1^T P D^-1 = 1).
 - qk for a head-pair runs as one packed matmul; biases and the 1/sqrt(K)
   scale ride an appended ones-row of combT.
 - leaky_relu on ACT (Prelu, alpha=0.2) / DVE (tunable split); exp on ACT;
   the adjacency mask is multiplicative 0/1 bf16 after exp (2x DVE mode).
 - Numerator + denominator + channel-128 accumulate in PSUM across the 8
   key tiles per head; normalization uses reciprocal_approx_fast + gpsimd
   partition_broadcast, folded per head-pair to keep PSUM inside 8 banks.
 - Hypernetwork uses the bilinear z-trick: zT[(i,d), m] = selT[i,m]*qvT[d,m]
   built by bf16 2x DVE TTs against host-prebroadcast qb rows; the whole
   per-query einsum is then 34 PSUM-accumulated matmuls per gate pair
   (bias and c128-channel terms are two more accumulating matmuls).
"""

import sys

sys.path.insert(0, "/opt/trn_rl_repo")

from contextlib import ExitStack

import numpy as np
import ml_dtypes

import concourse.bass as bass
import concourse.bacc as bacc
import concourse.tile as tile
from concourse import mybir
from concourse import bass_isa
from concourse.bass_utils import run_bass_kernel_spmd
from concourse.masks import make_identity
from concourse.bass import ts

P = 128
B, N, D, H, QV = 8, 1024, 64, 4, 32
C = 2 * D + 1            # 129
KD = C // 8              # 16
NQ = 512                 # selected nodes (queries) per graph
F32 = mybir.dt.float32
BF16 = mybir.dt.bfloat16
AX = mybir.AxisListType
ALU = mybir.AluOpType
ACTF = mybir.ActivationFunctionType

NT = N // P              # 8 key tiles

# leaky on ACT (Prelu) for the first ACT_LEAKY of 16 (pair, tile) units;
# the rest use a DVE STT. Sim cannot run Prelu -> test.py flips
# USE_LRELU[0]=False to force the DVE path everywhere.
USE_LRELU = [True]
ACT_LEAKY = 12


def build_graph(hw_leaky=True):
    nc = bacc.Bacc()

    combT_d = nc.declare_dram_parameter("combT", [P, N], BF16, False)
    c128_d = nc.declare_dram_parameter("c128", [1, N], BF16, False)
    kqw_d = nc.declare_dram_parameter("kqw", [P, 256], BF16, False)
    kqwL_d = nc.declare_dram_parameter("kqwL", [1, 256], BF16, False)
    bkq_d = nc.declare_dram_parameter("bkq", [KD, 2 * H], F32, False)
    wv1a_d = nc.declare_dram_parameter("wv1a", [P, 516], BF16, False)
    wv1b_d = nc.declare_dram_parameter("wv1b", [1, 516], BF16, False)
    adjT_d = nc.declare_dram_parameter("adjT", [P, NT * NQ], BF16, False)
    w2a_d = nc.declare_dram_parameter("w2a", [P, C], BF16, False)
    w2b_d = nc.declare_dram_parameter("w2b", [1, C], BF16, False)
    bias_d = nc.declare_dram_parameter("biases", [P, 4], F32, False)
    biasL_d = nc.declare_dram_parameter("biasesL", [1, 4], F32, False)
    qvT_d = nc.declare_dram_parameter("qvT", [QV, NQ], BF16, False)
    qb_d = nc.declare_dram_parameter("qb", [P, QV * NQ], BF16, False)
    wzru_d = nc.declare_dram_parameter("wzru", [P, QV * P], BF16, False)
    wzc_d = nc.declare_dram_parameter("wzc", [P, QV * D], BF16, False)
    # [32, 384]: cols 0:128 wzruL, 128:256 bru, 256:320 wzcL, 320:384 bc
    small_d = nc.declare_dram_parameter("smalls", [QV, 384], BF16, False)
    out_d = nc.declare_dram_parameter("out", [NQ, D], F32, True)

    with tile.TileContext(nc) as tc, ExitStack() as ctx:
        sing = ctx.enter_context(tc.tile_pool(name="sing", bufs=1))
        smp = ctx.enter_context(tc.tile_pool(name="smp", bufs=3))
        pep = ctx.enter_context(tc.tile_pool(name="pep", bufs=4))
        work = ctx.enter_context(tc.tile_pool(name="work", bufs=3))
        # PSUM budget (8 banks): psS 2x[128,1024] = 4, psY 2x[128,512] = 2,
        # psE 2x[2,512] = 2.  qk/V/MLP/hyper psums share these pools.
        psS = ctx.enter_context(tc.tile_pool(name="psS", bufs=2, space="PSUM"))
        psY = ctx.enter_context(tc.tile_pool(name="psY", bufs=2, space="PSUM"))
        psE = ctx.enter_context(tc.tile_pool(name="psE", bufs=2, space="PSUM"))

        identf = sing.tile([P, P], F32)
        make_identity(nc, identf[:])
        zeroN = sing.tile([KD, N], F32, name="zeroN")
        nc.gpsimd.memset(zeroN[:], 0.0)

        # ---------------- input DMAs (sync queue, rough use order) --------
        combT = sing.tile([P, N], BF16)
        nc.sync.dma_start(combT[:], combT_d[:, :])
        cxr = sing.tile([1, N], BF16)            # channel-128 row (last x)
        nc.sync.dma_start(cxr[:], c128_d[:, :])
        kqw = sing.tile([P, 256], BF16)
        kqwL = sing.tile([1, 256], BF16)
        bkq = sing.tile([KD, 2 * H], F32)
        nc.sync.dma_start(kqw[:], kqw_d[:, :])
        nc.sync.dma_start(kqwL[:], kqwL_d[:, :])
        nc.sync.dma_start(bkq[:], bkq_d[:, :])
        wv1a = sing.tile([P, 516], BF16)
        wv1b = sing.tile([1, 516], BF16)
        nc.sync.dma_start(wv1a[:], wv1a_d[:, :])
        nc.sync.dma_start(wv1b[:], wv1b_d[:, :])
        adjT = sing.tile([P, NT * NQ], BF16)
        nc.scalar.dma_start(adjT[:], adjT_d[:, :])
        w2a = sing.tile([P, C], BF16)
        w2b = sing.tile([1, C], BF16)
        nc.sync.dma_start(w2a[:], w2a_d[:, :])
        nc.sync.dma_start(w2b[:], w2b_d[:, :])
        biases = sing.tile([P, 4], F32)   # cols: 0 = b1, 1 = b2
        biasesL = sing.tile([1, 4], F32)
        nc.sync.dma_start(biases[:], bias_d[:, :])
        nc.sync.dma_start(biasesL[:], biasL_d[:, :])
        qvT = sing.tile([QV, NQ], BF16)
        nc.sync.dma_start(qvT[:], qvT_d[:, :])
        qb = sing.tile([P, QV * NQ], BF16)
        nc.scalar.dma_start(qb[:], qb_d[:, :])
        wzru = sing.tile([P, QV * P], BF16)
        nc.sync.dma_start(wzru[:], wzru_d[:, :])
        wzc = sing.tile([P, QV * D], BF16)
        nc.sync.dma_start(wzc[:], wzc_d[:, :])
        smalls = sing.tile([QV, 384], BF16)
        nc.sync.dma_start(smalls[:], small_d[:, :])

        # ---------------- qk: per head-pair packed matmul -----------------
        # psum rows per pair: [k_h0(16)@0 .. q_h0(16)@32 .. k_h1@64 q_h1@96]
        kT = [sing.tile([KD, N], BF16, tag=f"kT{h}", name=f"kT{h}")
              for h in range(H)]
        qT = [sing.tile([KD, NQ], BF16, tag=f"qT{h}", name=f"qT{h}")
              for h in range(H)]
        for p_ in range(2):
            ps = psS.tile([P, N], F32, tag="ps", name="qk")
            for half in range(2):
                nc.tensor.matmul(ps[:, ts(half, NQ)], kqw[:, ts(p_, P)],
                                 combT[:, ts(half, NQ)], start=True, stop=False)
                nc.tensor.matmul(ps[:, ts(half, NQ)], kqwL[:, ts(p_, P)],
                                 cxr[:, ts(half, NQ)], start=False, stop=True)
            for hh in range(2):
                h = 2 * p_ + hh
                nc.vector.scalar_tensor_tensor(
                    kT[h][:], ps[64 * hh:64 * hh + KD, :], bkq[:, h:h + 1],
                    zeroN[0:KD, :], op0=ALU.add, op1=ALU.add)
                nc.vector.scalar_tensor_tensor(
                    qT[h][:], ps[64 * hh + 32:64 * hh + 48, 0:NQ],
                    bkq[:, H + h:H + h + 1], zeroN[0:KD, 0:NQ],
                    op0=ALU.add, op1=ALU.add)

        # ---------------- V phase: U = comb @ (Wv W1), all heads ----------
        vt = [sing.tile([P, H, 130], BF16, tag=f"vt{i}", name=f"vt{i}")
              for i in range(NT)]
        for i in range(NT):
            pv = psS.tile([P, N], F32, tag="ps", name="pv")
            for g, off in ((0, 0), (1, NQ)):
                nc.tensor.matmul(pv[:, off:off + 258], combT[:, ts(i, P)],
                                 wv1a[:, g * 258:(g + 1) * 258],
                                 start=True, stop=False)
                nc.tensor.matmul(pv[:, off:off + 258], cxr[0:1, ts(i, P)],
                                 wv1b[:, g * 258:(g + 1) * 258],
                                 start=False, stop=True)
            nc.vector.tensor_copy(
                vt[i][:, :, 0:129].rearrange("p (a h) c -> p a h c", a=2),
                pv[:].rearrange("p (a b) -> p a b", b=NQ)[:, :, 0:258]
                .rearrange("p a (h c) -> p a h c", c=129))
            nc.gpsimd.memset(vt[i][:, :, 129:130], 1.0)

        # ---------------- attention + per-pair softmax norm ---------------
        m1acc = sing.tile([P, NQ], F32, name="m1acc")
        crs = [sing.tile([2, NQ], F32, tag=f"crs{p_}", name=f"crs{p_}")
               for p_ in range(2)]
        unit = 0
        for p_ in range(2):
            Y0 = psY.tile([P, NQ], F32, tag="Y", name="Y0")
            Y1 = psY.tile([P, NQ], F32, tag="Y", name="Y1")
            E0 = psE.tile([2, NQ], F32, tag="E", name="E0")
            E1 = psE.tile([2, NQ], F32, tag="E", name="E1")
            h0, h1 = 2 * p_, 2 * p_ + 1
            pes = {}

            def accum(i):
                pe = pes.pop(i)
                st, sp = i == 0, i == NT - 1
                nc.tensor.matmul(Y0[:], vt[i][:, h0, 0:P], pe[:, 0:NQ],
                                 start=st, stop=sp)
                nc.tensor.matmul(E0[:], vt[i][:, h0, P:130], pe[:, 0:NQ],
                                 start=st, stop=sp)
                nc.tensor.matmul(Y1[:], vt[i][:, h1, 0:P], pe[:, NQ:N],
                                 start=st, stop=sp)
                nc.tensor.matmul(E1[:], vt[i][:, h1, P:130], pe[:, NQ:N],
                                 start=st, stop=sp)

            for i in range(NT):
                ps = psS.tile([P, N], F32, tag="ps", name="sc")
                nc.tensor.matmul(ps[:, 0:NQ], kT[h0][:, ts(i, P)], qT[h0][:],
                                 start=True, stop=True)
                nc.tensor.matmul(ps[:, NQ:N], kT[h1][:, ts(i, P)], qT[h1][:],
                                 start=True, stop=True)
                sm = smp.tile([P, N], BF16, tag="sm", name="sm")
                if hw_leaky:
                    nc.scalar.activation(sm[:], ps[:], ACTF.Prelu, alpha=0.2)
                else:
                    nc.vector.scalar_tensor_tensor(sm[:], ps[:], 0.2, ps[:],
                                                   op0=ALU.mult, op1=ALU.max)
                unit += 1
                pe = pep.tile([P, N], BF16, tag="pe", name="pe")
                nc.scalar.activation(pe[:], sm[:], ACTF.Exp)
                nc.vector.tensor_tensor(
                    pe[:].rearrange("p (a b) -> p a b", b=NQ),
                    pe[:].rearrange("p (a b) -> p a b", b=NQ),
                    adjT[:, None, ts(i, NQ)].to_broadcast((P, 2, NQ)),
                    ALU.mult)
                pes[i] = pe
                # software pipeline: numerator matmuls run 2 tiles behind the
                # scores so the in-order PE queue never waits on prelu/exp.
                if i >= 2:
                    accum(i - 2)
            accum(NT - 2)
            accum(NT - 1)
            # per-pair normalization (frees Y/E psums for the next pair)
            ex0 = work.tile([2, NQ], F32, tag="ex", name="ex0")
            ex1 = work.tile([2, NQ], F32, tag="ex", name="ex1")
            nc.vector.tensor_copy(ex0[:], E0[:])
            nc.vector.tensor_copy(ex1[:], E1[:])
            dc = sing.tile([2, NQ], F32, tag=f"dc{p_}", name=f"dc{p_}")
            cc = sing.tile([2, NQ], F32, tag=f"cc{p_}", name=f"cc{p_}")
            nc.gpsimd.dma_start(dc[0:1, :], ex0[1:2, :])
            nc.gpsimd.dma_start(dc[1:2, :], ex1[1:2, :])
            nc.gpsimd.dma_start(cc[0:1, :], ex0[0:1, :])
            nc.gpsimd.dma_start(cc[1:2, :], ex1[0:1, :])
            rinv = sing.tile([2, NQ], F32, tag=f"ri{p_}", name=f"ri{p_}")
            nc.vector.reciprocal_approx_fast(rinv[:], dc[:])
            nc.vector.tensor_tensor(crs[p_][:], cc[:], rinv[:], ALU.mult)
            ri1 = sing.tile([1, NQ], F32, tag=f"ri1{p_}", name=f"ri1{p_}")
            nc.gpsimd.dma_start(ri1[:], rinv[1:2, :])
            for hh, Yh in ((0, Y0), (1, Y1)):
                rb = work.tile([P, NQ], F32, tag="rb", name="rb")
                nc.gpsimd.partition_broadcast(
                    rb[:], rinv[0:1, :] if hh == 0 else ri1[:])
                if p_ == 0 and hh == 0:
                    nc.vector.tensor_tensor(m1acc[:], Yh[:], rb[:], ALU.mult)
                else:
                    t_ = work.tile([P, NQ], F32, tag="nt", name="nt")
                    nc.vector.tensor_tensor(t_[:], Yh[:], rb[:], ALU.mult)
                    nc.vector.tensor_tensor(m1acc[:], m1acc[:], t_[:], ALU.add)

        # ---------------- MLP channel 128 + relu + W2 + residual ----------
        c4s = sing.tile([2, NQ], F32, name="c4s")
        nc.vector.tensor_tensor(c4s[:], crs[0][:], crs[1][:], ALU.add)
        nc.gpsimd.partition_all_reduce(c4s[:], c4s[:], 2, bass_isa.ReduceOp.add)
        m1T = sing.tile([P, NQ], BF16, name="m1T")
        nc.scalar.activation(m1T[:], m1acc[:], ACTF.Relu, bias=biases[:, 0:1])
        m1L = sing.tile([1, NQ], BF16, name="m1L")
        nc.scalar.activation(m1L[:], c4s[0:1, :], ACTF.Relu,
                             bias=biasesL[0:1, 0:1])
        pcf = psY.tile([P, NQ], F32, tag="Y", name="pcf")
        nc.tensor.matmul(pcf[:], w2a[:, 0:P], m1T[:], start=True, stop=False)
        nc.tensor.matmul(pcf[:], w2b[:, 0:P], m1L[:], start=False, stop=True)
        pcfL = psE.tile([2, NQ], F32, tag="E", name="pcfL")
        nc.tensor.matmul(pcfL[0:1, :], w2a[:, P:C], m1T[:], start=True, stop=False)
        nc.tensor.matmul(pcfL[0:1, :], w2b[:, P:C], m1L[:], start=False, stop=True)
        cf = sing.tile([P, NQ], BF16, name="cf")
        nc.vector.scalar_tensor_tensor(cf[:], pcf[:], biases[:, 1:2],
                                       combT[:, 0:NQ], op0=ALU.add, op1=ALU.add)
        cl_row = sing.tile([1, NQ], BF16, name="cl_row")
        nc.vector.scalar_tensor_tensor(cl_row[:], pcfL[0:1, :],
                                       biasesL[0:1, 1:2], cxr[0:1, 0:NQ],
                                       op0=ALU.add, op1=ALU.add)

        # preload the sigmoid/tanh table during the MLP window
        scrap = sing.tile([1, 1], F32, name="scrap")
        nc.scalar.activation(scrap[:], biasesL[0:1, 0:1], ACTF.Sigmoid)

        # ---------------- hypernetwork (z-trick, all transposed) ----------
        zq = sing.tile([P, QV * NQ], BF16, name="zq")

        def build_z(selT):
            for g in range(8):
                nc.vector.tensor_tensor(
                    zq[:, g * 2048:(g + 1) * 2048].rearrange(
                        "p (a b) -> p a b", b=NQ),
                    selT[:, None, 0:NQ].to_broadcast((P, 4, NQ)),
                    qb[:, g * 2048:(g + 1) * 2048].rearrange(
                        "p (a b) -> p a b", b=NQ),
                    ALU.mult)

        def hyper_mms(pdst, nr, wz, wL, bL, qcl_):
            for d in range(QV):
                g, dd = d // 4, d % 4
                nc.tensor.matmul(pdst, wz[:, d * nr:(d + 1) * nr],
                                 zq[:, g * 2048 + dd * NQ:
                                    g * 2048 + (dd + 1) * NQ],
                                 start=(d == 0), stop=False)
            nc.tensor.matmul(pdst, wL, qcl_[:], start=False, stop=False)
            nc.tensor.matmul(pdst, bL, qvT[:], start=False, stop=True)

        # r | u
        rep_cl = sing.tile([QV, NQ], BF16, name="rep_cl")
        nc.gpsimd.partition_broadcast(rep_cl[:], cl_row[:])
        qcl = sing.tile([QV, NQ], BF16, name="qcl")
        nc.vector.tensor_tensor(qcl[:], qvT[:], rep_cl[:], ALU.mult)
        build_z(cf)
        pru = psY.tile([P, NQ], F32, tag="Y", name="pru")
        hyper_mms(pru[:], P, wzru, smalls[:, 0:P], smalls[:, P:256], qcl)
        ru = sing.tile([P, NQ], BF16, name="ru")
        nc.scalar.activation(ru[:], pru[:], ACTF.Sigmoid)

        # selc = [hn (rows 0:64) | x (rows 64:128)], channel 128 = cx2 row 0
        selc = sing.tile([P, NQ], BF16, name="selc")
        nc.vector.tensor_tensor(selc[0:D, :], ru[0:D, :], combT[0:D, 0:NQ],
                                ALU.mult)
        nc.vector.tensor_copy(selc[D:P, :], combT[D:P, 0:NQ])
        rep_c2 = sing.tile([QV, NQ], BF16, name="rep_c2")
        nc.gpsimd.partition_broadcast(rep_c2[:], cxr[0:1, 0:NQ])
        qcl2 = sing.tile([QV, NQ], BF16, name="qcl2")
        nc.vector.tensor_tensor(qcl2[:], qvT[:], rep_c2[:], ALU.mult)
        build_z(selc)
        pc = psY.tile([P, NQ], F32, tag="Y", name="pc")
        hyper_mms(pc[0:D, :], D, wzc, smalls[:, 256:320], smalls[:, 320:384],
                  qcl2)
        cand = sing.tile([D, NQ], BF16, name="cand")
        nc.scalar.activation(cand[:], pc[0:D, :], ACTF.Tanh)

        # out = hn + u*(cand - hn)   (hn = selc rows 0:64, u = ru rows 64:128)
        u64 = sing.tile([D, NQ], BF16, name="u64")
        nc.vector.tensor_copy(u64[:], ru[D:P, :])
        d1 = sing.tile([D, NQ], BF16, name="d1")
        nc.vector.tensor_tensor(d1[:], cand[:], selc[0:D, :], ALU.subtract)
        nc.vector.tensor_tensor(d1[:], d1[:], u64[:], ALU.mult)
        outT = sing.tile([D, NQ], F32, name="outT")
        nc.vector.tensor_tensor(outT[:], d1[:], selc[0:D, :], ALU.add)
        for j in range(4):
            pt = psY.tile([P, D], F32, tag="Y", name="pt")
            nc.tensor.transpose(pt[:, 0:D], outT[:, ts(j, P)],
                                identf[0:D, 0:D])
            ob = work.tile([P, D], F32, tag="ob", name="ob")
            nc.vector.tensor_copy(ob[:], pt[:, 0:D])
            nc.sync.dma_start(out_d[ts(j, P), :], ob[:])

    return nc


_NC_CACHE = {}


def _get_nc():
    key = bool(USE_LRELU[0])
    if key not in _NC_CACHE:
        nc = build_graph(hw_leaky=key)
        if not nc.is_finalized():
            nc.finalize()
        _NC_CACHE[key] = nc
    return _NC_CACHE[key]


# channel reorder: new order = [h (64) | x (65)]
_R = np.concatenate([np.arange(65, 129), np.arange(0, 65)])
_BF = ml_dtypes.bfloat16


def _bf(a):
    return np.ascontiguousarray(np.asarray(a, np.float32).astype(_BF))


def _prep_shared(Wq, bq, Wk, bk, Wv, bv, W1, b1, W2, b2, Wr, br, Wu, bu, Wc, bc):
    f32 = np.float32
    Wq, bq = np.asarray(Wq, f32), np.asarray(bq, f32)
    Wk, bk = np.asarray(Wk, f32), np.asarray(bk, f32)
    Wv, bv = np.asarray(Wv, f32), np.asarray(bv, f32)
    W1, b1 = np.asarray(W1, f32).reshape(H, C, C), np.asarray(b1, f32)
    W2, b2 = np.asarray(W2, f32), np.asarray(b2, f32)

    # qk packed: per pair cols [k_h0|0|q_h0/4|0|k_h1|0|q_h1/4|0] (16 each);
    # contraction rows = 128 reordered channels + c128; biases ride the
    # psum->sbuf copies as per-partition bias APs (bkq).
    Wq_r, Wk_r = Wq[:, _R, :], Wk[:, _R, :]
    kqw = np.zeros((C, 256), f32)
    bkq = np.zeros((KD, 2 * H), f32)
    for h in range(H):
        base = (h // 2) * 128 + (h % 2) * 64
        kqw[0:129, base:base + 16] = Wk_r[h]
        kqw[0:129, base + 32:base + 48] = Wq_r[h] * 0.25
        bkq[:, h] = bk[h]
        bkq[:, H + h] = bq[h] * 0.25

    # V with W1 folded: U_h = Wv_h @ W1_h, contraction rows reordered
    U = np.stack([(Wv[h] @ W1[h])[_R] for h in range(H)])    # [H, 129, 129]
    wv1 = np.ascontiguousarray(
        np.transpose(U, (1, 0, 2)).reshape(C, H * C))        # [129, 516]
    b1_eff = b1 + sum(bv[h] @ W1[h] for h in range(H))

    w2r = W2[:, _R]                                          # cols reordered
    b2r = b2[_R]
    biases = np.zeros((C, 4), f32)
    biases[:, 0] = b1_eff
    biases[:, 1] = b2r

    Wr_r = np.asarray(Wr, f32)[:, _R, :]
    Wu_r = np.asarray(Wu, f32)[:, _R, :]
    Wc_r = np.asarray(Wc, f32)[:, _R, :]
    wzru = np.ascontiguousarray(np.transpose(
        np.concatenate([Wr_r[:, 0:128, :], Wu_r[:, 0:128, :]], 2),
        (1, 0, 2)).reshape(P, QV * P))
    wzc = np.ascontiguousarray(
        np.transpose(Wc_r[:, 0:128, :], (1, 0, 2)).reshape(P, QV * D))
    smalls = np.zeros((QV, 384), f32)
    smalls[:, 0:64] = Wr_r[:, 128, :]
    smalls[:, 64:128] = Wu_r[:, 128, :]
    smalls[:, 128:192] = np.asarray(br, f32)
    smalls[:, 192:256] = np.asarray(bu, f32)
    smalls[:, 256:320] = Wc_r[:, 128, :]
    smalls[:, 320:384] = np.asarray(bc, f32)

    return dict(
        kqw=_bf(kqw[0:128]), kqwL=_bf(kqw[128:129]),
        bkq=np.ascontiguousarray(bkq),
        wv1a=_bf(wv1[0:128]), wv1b=_bf(wv1[128:129]),
        w2a=_bf(w2r[0:128]), w2b=_bf(w2r[128:129]),
        biases=np.ascontiguousarray(biases[0:128]),
        biasesL=np.ascontiguousarray(biases[128:129]),
        wzru=_bf(wzru), wzc=_bf(wzc), smalls=_bf(smalls),
    )


def _prep_core(b, x, h, query_vectors, adj, nodes_n, shared):
    idx = nodes_n[b * NQ:(b + 1) * NQ].astype(np.int64)
    rest = np.setdiff1d(np.arange(N, dtype=np.int64), idx)
    perm = np.concatenate([idx, rest])
    comb = np.concatenate([x[b][perm], h[b][perm]], 1)[:, _R]  # [N,129] reord
    combT = np.ascontiguousarray(comb.T.astype(_BF))           # [129, N]
    qv = query_vectors[b * NQ:(b + 1) * NQ]                    # [512, 32]
    qvT = np.ascontiguousarray(qv.T.astype(_BF))               # [32, 512]
    # qb [128, 32*512]: qb[p, d*512 + m] = qv[m, d]  (uint16 view = fast)
    qvT_u16 = qvT.view(np.uint16)
    qb = np.ascontiguousarray(
        np.broadcast_to(qvT_u16[None, :, :], (P, QV, NQ))
        .reshape(P, QV * NQ)).view(_BF)
    adj01 = (adj[np.ix_(idx, perm)] != 0).astype(np.float32).T  # [N, 512]
    adjT = np.ascontiguousarray(
        adj01.reshape(NT, P, NQ).transpose(1, 0, 2).reshape(P, NT * NQ)
    ).astype(_BF)
    d = dict(shared)
    d["combT"] = np.ascontiguousarray(combT[0:128])
    d["c128"] = np.ascontiguousarray(combT[128:129])
    d["qvT"] = qvT
    d["qb"] = qb
    d["adjT"] = adjT
    return d


def make_in_maps(x, h, query_vectors, adj, nodes_b, nodes_n, **weights):
    x = np.asarray(x, np.float32)
    h = np.asarray(h, np.float32)
    query_vectors = np.asarray(query_vectors, np.float32)
    adj = np.asarray(adj)
    nodes_n = np.asarray(nodes_n)
    shared = _prep_shared(**weights)
    return [_prep_core(b, x, h, query_vectors, adj, nodes_n, shared)
            for b in range(B)]


def kernel(x, h, query_vectors, adj, nodes_b, nodes_n,
           Wq, bq, Wk, bk, Wv, bv, W1, b1, W2, b2,
           Wr, br, Wu, bu, Wc, bc):
    in_maps = make_in_maps(
        x, h, query_vectors, adj, nodes_b, nodes_n,
        Wq=Wq, bq=bq, Wk=Wk, bk=bk, Wv=Wv, bv=bv, W1=W1, b1=b1, W2=W2, b2=b2,
        Wr=Wr, br=br, Wu=Wu, bu=bu, Wc=Wc, bc=bc)
    nc = _get_nc()
    res = run_bass_kernel_spmd(nc, in_maps, list(range(B)))
    outs = [np.asarray(res.results[b]["out"], np.float32) for b in range(B)]
    return np.concatenate(outs, axis=0)


# revision 32
# speedup vs baseline: 1.1014x; 1.1014x over previous
"""AGATCellWithMLP Trainium2 kernel: 8-core data-parallel over batch B.

v2 design (one graph per core, everything transposed [channel, node]):
 - Host sends combT pre-transposed bf16 with channels reordered [h | x] so
   that every on-chip partition slice lands on a 0/32/64 base.
 - W1 is folded into Wv host-side (U_h = Wv_h @ W1_h), so the attention
   numerator matmuls directly produce the MLP hidden pre-activations; the
   bv@W1 term is exact via the softmax denominator (1^T P D^-1 = 1).
 - qk for a head-pair runs as one packed matmul; biases and the 1/sqrt(K)
   scale ride an appended ones-row of combT.
 - leaky_relu on ACT (Prelu, alpha=0.2) / DVE (tunable split); exp on ACT;
   the adjacency mask is multiplicative 0/1 bf16 after exp (2x DVE mode).
 - Numerator + denominator + channel-128 accumulate in PSUM across the 8
   key tiles per head; normalization uses reciprocal_approx_fast + gpsimd
   partition_broadcast, folded per head-pair to keep PSUM inside 8 banks.
 - Hypernetwork uses the bilinear z-trick: zT[(i,d), m] = selT[i,m]*qvT[d,m]
   built by bf16 2x DVE TTs against host-prebroadcast qb rows; the whole
   per-query einsum is then 34 PSUM-accumulated matmuls per gate pair
   (bias and c128-channel terms are two more accumulating matmuls).
"""

import sys

sys.path.insert(0, "/opt/trn_rl_repo")

from contextlib import ExitStack

import numpy as np
import ml_dtypes

import concourse.bass as bass
import concourse.bacc as bacc
import concourse.tile as tile
from concourse import mybir
from concourse import bass_isa
from concourse.bass_utils import run_bass_kernel_spmd
from concourse.masks import make_identity
from concourse.bass import ts

P = 128
B, N, D, H, QV = 8, 1024, 64, 4, 32
C = 2 * D + 1            # 129
KD = C // 8              # 16
NQ = 512                 # selected nodes (queries) per graph
F32 = mybir.dt.float32
BF16 = mybir.dt.bfloat16
AX = mybir.AxisListType
ALU = mybir.AluOpType
ACTF = mybir.ActivationFunctionType

NT = N // P              # 8 key tiles

# leaky on ACT (Prelu) for the first ACT_LEAKY of 16 (pair, tile) units;
# the rest use a DVE STT. Sim cannot run Prelu -> test.py flips
# USE_LRELU[0]=False to force the DVE path everywhere.
USE_LRELU = [True]
ACT_LEAKY = 12


def build_graph(hw_leaky=True):
    nc = bacc.Bacc()

    combT_d = nc.declare_dram_parameter("combT", [P, N], BF16, False)
    c128_d = nc.declare_dram_parameter("c128", [1, N], BF16, False)
    kqw_d = nc.declare_dram_parameter("kqw", [P, 256], BF16, False)
    kqwL_d = nc.declare_dram_parameter("kqwL", [1, 256], BF16, False)
    bkq_d = nc.declare_dram_parameter("bkq", [KD, 2 * H], F32, False)
    wv1a_d = nc.declare_dram_parameter("wv1a", [P, 516], BF16, False)
    wv1b_d = nc.declare_dram_parameter("wv1b", [1, 516], BF16, False)
    adjT_d = nc.declare_dram_parameter("adjT", [P, NT * NQ], BF16, False)
    w2a_d = nc.declare_dram_parameter("w2a", [P, C], BF16, False)
    w2b_d = nc.declare_dram_parameter("w2b", [1, C], BF16, False)
    bias_d = nc.declare_dram_parameter("biases", [P, 4], F32, False)
    biasL_d = nc.declare_dram_parameter("biasesL", [1, 4], F32, False)
    qvT_d = nc.declare_dram_parameter("qvT", [QV, NQ], BF16, False)
    qb_d = nc.declare_dram_parameter("qb", [P, QV * NQ], BF16, False)
    wzru_d = nc.declare_dram_parameter("wzru", [P, QV * P], BF16, False)
    wzc_d = nc.declare_dram_parameter("wzc", [P, QV * D], BF16, False)
    # [32, 384]: cols 0:128 wzruL, 128:256 bru, 256:320 wzcL, 320:384 bc
    small_d = nc.declare_dram_parameter("smalls", [QV, 384], BF16, False)
    out_d = nc.declare_dram_parameter("out", [NQ, D], F32, True)

    with tile.TileContext(nc) as tc, ExitStack() as ctx:
        sing = ctx.enter_context(tc.tile_pool(name="sing", bufs=1))
        smp = ctx.enter_context(tc.tile_pool(name="smp", bufs=3))
        pep = ctx.enter_context(tc.tile_pool(name="pep", bufs=4))
        work = ctx.enter_context(tc.tile_pool(name="work", bufs=3))
        # PSUM budget (8 banks): psS 2x[128,1024] = 4, psY 2x[128,512] = 2,
        # psE 2x[2,512] = 2.  qk/V/MLP/hyper psums share these pools.
        psS = ctx.enter_context(tc.tile_pool(name="psS", bufs=2, space="PSUM"))
        psY = ctx.enter_context(tc.tile_pool(name="psY", bufs=2, space="PSUM"))
        psE = ctx.enter_context(tc.tile_pool(name="psE", bufs=2, space="PSUM"))

        identf = sing.tile([P, P], F32)
        make_identity(nc, identf[:])
        zeroN = sing.tile([KD, N], F32, name="zeroN")
        nc.gpsimd.memset(zeroN[:], 0.0)

        # ---------------- input DMAs (sync queue, rough use order) --------
        combT = sing.tile([P, N], BF16)
        nc.sync.dma_start(combT[:], combT_d[:, :])
        cxr = sing.tile([1, N], BF16)            # channel-128 row (last x)
        nc.sync.dma_start(cxr[:], c128_d[:, :])
        kqw = sing.tile([P, 256], BF16)
        kqwL = sing.tile([1, 256], BF16)
        bkq = sing.tile([KD, 2 * H], F32)
        nc.sync.dma_start(kqw[:], kqw_d[:, :])
        nc.sync.dma_start(kqwL[:], kqwL_d[:, :])
        nc.sync.dma_start(bkq[:], bkq_d[:, :])
        wv1a = sing.tile([P, 516], BF16)
        wv1b = sing.tile([1, 516], BF16)
        nc.sync.dma_start(wv1a[:], wv1a_d[:, :])
        nc.sync.dma_start(wv1b[:], wv1b_d[:, :])
        adjT = sing.tile([P, NT * NQ], BF16)
        nc.sync.dma_start(adjT[:], adjT_d[:, :])
        w2a = sing.tile([P, C], BF16)
        w2b = sing.tile([1, C], BF16)
        nc.sync.dma_start(w2a[:], w2a_d[:, :])
        nc.sync.dma_start(w2b[:], w2b_d[:, :])
        biases = sing.tile([P, 4], F32)   # cols: 0 = b1, 1 = b2
        biasesL = sing.tile([1, 4], F32)
        nc.sync.dma_start(biases[:], bias_d[:, :])
        nc.sync.dma_start(biasesL[:], biasL_d[:, :])
        qvT = sing.tile([QV, NQ], BF16)
        nc.sync.dma_start(qvT[:], qvT_d[:, :])
        # big hyper-stage prefetches are issued AFTER the V phase below so
        # their transfers queue behind the startup-critical DMAs above
        qb = sing.tile([P, QV * NQ], BF16)
        wzru = sing.tile([P, QV * P], BF16)
        wzc = sing.tile([P, QV * D], BF16)
        smalls = sing.tile([QV, 384], BF16)

        # ---------------- qk: per head-pair packed matmul -----------------
        # psum rows per pair: [k_h0(16)@0 .. q_h0(16)@32 .. k_h1@64 q_h1@96]
        kT = [sing.tile([KD, N], BF16, tag=f"kT{h}", name=f"kT{h}")
              for h in range(H)]
        qT = [sing.tile([KD, NQ], BF16, tag=f"qT{h}", name=f"qT{h}")
              for h in range(H)]
        for p_ in range(2):
            ps = psS.tile([P, N], F32, tag="ps", name="qk")
            for half in range(2):
                nc.tensor.matmul(ps[:, ts(half, NQ)], kqw[:, ts(p_, P)],
                                 combT[:, ts(half, NQ)], start=True, stop=False)
                nc.tensor.matmul(ps[:, ts(half, NQ)], kqwL[:, ts(p_, P)],
                                 cxr[:, ts(half, NQ)], start=False, stop=True)
            for hh in range(2):
                h = 2 * p_ + hh
                nc.vector.scalar_tensor_tensor(
                    kT[h][:], ps[64 * hh:64 * hh + KD, :], bkq[:, h:h + 1],
                    zeroN[0:KD, :], op0=ALU.add, op1=ALU.add)
                nc.vector.scalar_tensor_tensor(
                    qT[h][:], ps[64 * hh + 32:64 * hh + 48, 0:NQ],
                    bkq[:, H + h:H + h + 1], zeroN[0:KD, 0:NQ],
                    op0=ALU.add, op1=ALU.add)

        # ---------------- V phase: U = comb @ (Wv W1), all heads ----------
        vt = [sing.tile([P, H, 130], BF16, tag=f"vt{i}", name=f"vt{i}")
              for i in range(NT)]
        for i in range(NT):
            pv = psS.tile([P, N], F32, tag="ps", name="pv")
            for g, off in ((0, 0), (1, NQ)):
                nc.tensor.matmul(pv[:, off:off + 258], combT[:, ts(i, P)],
                                 wv1a[:, g * 258:(g + 1) * 258],
                                 start=True, stop=False)
                nc.tensor.matmul(pv[:, off:off + 258], cxr[0:1, ts(i, P)],
                                 wv1b[:, g * 258:(g + 1) * 258],
                                 start=False, stop=True)
            nc.vector.tensor_copy(
                vt[i][:, :, 0:129].rearrange("p (a h) c -> p a h c", a=2),
                pv[:].rearrange("p (a b) -> p a b", b=NQ)[:, :, 0:258]
                .rearrange("p a (h c) -> p a h c", c=129))
            nc.gpsimd.memset(vt[i][:, :, 129:130], 1.0)

        nc.sync.dma_start(qb[:], qb_d[:, :])
        nc.sync.dma_start(wzru[:], wzru_d[:, :])
        nc.sync.dma_start(wzc[:], wzc_d[:, :])
        nc.sync.dma_start(smalls[:], small_d[:, :])

        # ---------------- attention + per-pair softmax norm ---------------
        m1acc = sing.tile([P, NQ], F32, name="m1acc")
        crs = [sing.tile([2, NQ], F32, tag=f"crs{p_}", name=f"crs{p_}")
               for p_ in range(2)]
        unit = 0
        for p_ in range(2):
            Y0 = psY.tile([P, NQ], F32, tag="Y", name="Y0")
            Y1 = psY.tile([P, NQ], F32, tag="Y", name="Y1")
            E0 = psE.tile([2, NQ], F32, tag="E", name="E0")
            E1 = psE.tile([2, NQ], F32, tag="E", name="E1")
            h0, h1 = 2 * p_, 2 * p_ + 1
            pes = {}

            def accum(i):
                pe = pes.pop(i)
                st, sp = i == 0, i == NT - 1
                nc.tensor.matmul(Y0[:], vt[i][:, h0, 0:P], pe[:, 0:NQ],
                                 start=st, stop=sp)
                nc.tensor.matmul(E0[:], vt[i][:, h0, P:130], pe[:, 0:NQ],
                                 start=st, stop=sp)
                nc.tensor.matmul(Y1[:], vt[i][:, h1, 0:P], pe[:, NQ:N],
                                 start=st, stop=sp)
                nc.tensor.matmul(E1[:], vt[i][:, h1, P:130], pe[:, NQ:N],
                                 start=st, stop=sp)

            for i in range(NT):
                ps = psS.tile([P, N], F32, tag="ps", name="sc")
                nc.tensor.matmul(ps[:, 0:NQ], kT[h0][:, ts(i, P)], qT[h0][:],
                                 start=True, stop=True)
                nc.tensor.matmul(ps[:, NQ:N], kT[h1][:, ts(i, P)], qT[h1][:],
                                 start=True, stop=True)
                sm = smp.tile([P, N], BF16, tag="sm", name="sm")
                if hw_leaky:
                    nc.scalar.activation(sm[:], ps[:], ACTF.Prelu, alpha=0.2)
                else:
                    nc.vector.scalar_tensor_tensor(sm[:], ps[:], 0.2, ps[:],
                                                   op0=ALU.mult, op1=ALU.max)
                unit += 1
                pe = pep.tile([P, N], BF16, tag="pe", name="pe")
                nc.scalar.activation(pe[:], sm[:], ACTF.Exp)
                nc.vector.tensor_tensor(
                    pe[:].rearrange("p (a b) -> p a b", b=NQ),
                    pe[:].rearrange("p (a b) -> p a b", b=NQ),
                    adjT[:, None, ts(i, NQ)].to_broadcast((P, 2, NQ)),
                    ALU.mult)
                pes[i] = pe
                # software pipeline: numerator matmuls run 2 tiles behind the
                # scores so the in-order PE queue never waits on prelu/exp.
                if i >= 2:
                    accum(i - 2)
            accum(NT - 2)
            accum(NT - 1)
            # per-pair normalization (frees Y/E psums for the next pair)
            ex0 = work.tile([2, NQ], F32, tag="ex", name="ex0")
            ex1 = work.tile([2, NQ], F32, tag="ex", name="ex1")
            nc.vector.tensor_copy(ex0[:], E0[:])
            nc.vector.tensor_copy(ex1[:], E1[:])
            dc = sing.tile([2, NQ], F32, tag=f"dc{p_}", name=f"dc{p_}")
            cc = sing.tile([2, NQ], F32, tag=f"cc{p_}", name=f"cc{p_}")
            nc.gpsimd.dma_start(dc[0:1, :], ex0[1:2, :])
            nc.gpsimd.dma_start(dc[1:2, :], ex1[1:2, :])
            nc.gpsimd.dma_start(cc[0:1, :], ex0[0:1, :])
            nc.gpsimd.dma_start(cc[1:2, :], ex1[0:1, :])
            rinv = sing.tile([2, NQ], F32, tag=f"ri{p_}", name=f"ri{p_}")
            nc.vector.reciprocal_approx_fast(rinv[:], dc[:])
            nc.vector.tensor_tensor(crs[p_][:], cc[:], rinv[:], ALU.mult)
            ri1 = sing.tile([1, NQ], F32, tag=f"ri1{p_}", name=f"ri1{p_}")
            nc.gpsimd.dma_start(ri1[:], rinv[1:2, :])
            for hh, Yh in ((0, Y0), (1, Y1)):
                rb = work.tile([P, NQ], F32, tag="rb", name="rb")
                nc.gpsimd.partition_broadcast(
                    rb[:], rinv[0:1, :] if hh == 0 else ri1[:])
                if p_ == 0 and hh == 0:
                    nc.vector.tensor_tensor(m1acc[:], Yh[:], rb[:], ALU.mult)
                else:
                    t_ = work.tile([P, NQ], F32, tag="nt", name="nt")
                    nc.vector.tensor_tensor(t_[:], Yh[:], rb[:], ALU.mult)
                    nc.vector.tensor_tensor(m1acc[:], m1acc[:], t_[:], ALU.add)

        # ---------------- MLP channel 128 + relu + W2 + residual ----------
        c4s = sing.tile([2, NQ], F32, name="c4s")
        nc.vector.tensor_tensor(c4s[:], crs[0][:], crs[1][:], ALU.add)
        nc.gpsimd.partition_all_reduce(c4s[:], c4s[:], 2, bass_isa.ReduceOp.add)
        m1T = sing.tile([P, NQ], BF16, name="m1T")
        nc.scalar.activation(m1T[:], m1acc[:], ACTF.Relu, bias=biases[:, 0:1])
        m1L = sing.tile([1, NQ], BF16, name="m1L")
        nc.scalar.activation(m1L[:], c4s[0:1, :], ACTF.Relu,
                             bias=biasesL[0:1, 0:1])
        pcf = psY.tile([P, NQ], F32, tag="Y", name="pcf")
        nc.tensor.matmul(pcf[:], w2a[:, 0:P], m1T[:], start=True, stop=False)
        nc.tensor.matmul(pcf[:], w2b[:, 0:P], m1L[:], start=False, stop=True)
        pcfL = psE.tile([2, NQ], F32, tag="E", name="pcfL")
        nc.tensor.matmul(pcfL[0:1, :], w2a[:, P:C], m1T[:], start=True, stop=False)
        nc.tensor.matmul(pcfL[0:1, :], w2b[:, P:C], m1L[:], start=False, stop=True)
        cf = sing.tile([P, NQ], BF16, name="cf")
        nc.vector.scalar_tensor_tensor(cf[:], pcf[:], biases[:, 1:2],
                                       combT[:, 0:NQ], op0=ALU.add, op1=ALU.add)
        cl_row = sing.tile([1, NQ], BF16, name="cl_row")
        nc.vector.scalar_tensor_tensor(cl_row[:], pcfL[0:1, :],
                                       biasesL[0:1, 1:2], cxr[0:1, 0:NQ],
                                       op0=ALU.add, op1=ALU.add)

        # preload the sigmoid/tanh table during the MLP window
        scrap = sing.tile([1, 1], F32, name="scrap")
        nc.scalar.activation(scrap[:], biasesL[0:1, 0:1], ACTF.Sigmoid)

        # ---------------- hypernetwork (z-trick, all transposed) ----------
        zq = [sing.tile([P, 4 * NQ], BF16, tag=f"zq{g}", name=f"zq{g}")
              for g in range(8)]

        def build_z(selT):
            for g in range(8):
                nc.vector.tensor_tensor(
                    zq[g][:].rearrange("p (a b) -> p a b", b=NQ),
                    selT[:, None, 0:NQ].to_broadcast((P, 4, NQ)),
                    qb[:, g * 2048:(g + 1) * 2048].rearrange(
                        "p (a b) -> p a b", b=NQ),
                    ALU.mult)

        def hyper_mms(pdst, nr, wz, wL, bL, qcl_):
            for d in range(QV):
                g, dd = d // 4, d % 4
                nc.tensor.matmul(pdst, wz[:, d * nr:(d + 1) * nr],
                                 zq[g][:, dd * NQ:(dd + 1) * NQ],
                                 start=(d == 0), stop=False)
            nc.tensor.matmul(pdst, wL, qcl_[:], start=False, stop=False)
            nc.tensor.matmul(pdst, bL, qvT[:], start=False, stop=True)

        # r | u
        rep_cl = sing.tile([QV, NQ], BF16, name="rep_cl")
        nc.gpsimd.partition_broadcast(rep_cl[:], cl_row[:])
        qcl = sing.tile([QV, NQ], BF16, name="qcl")
        nc.vector.tensor_tensor(qcl[:], qvT[:], rep_cl[:], ALU.mult)
        build_z(cf)
        pru = psY.tile([P, NQ], F32, tag="Y", name="pru")
        hyper_mms(pru[:], P, wzru, smalls[:, 0:P], smalls[:, P:256], qcl)
        ru = sing.tile([P, NQ], BF16, name="ru")
        nc.scalar.activation(ru[:], pru[:], ACTF.Sigmoid)

        # selc = [hn (rows 0:64) | x (rows 64:128)], channel 128 = cx2 row 0
        selc = sing.tile([P, NQ], BF16, name="selc")
        nc.vector.tensor_tensor(selc[0:D, :], ru[0:D, :], combT[0:D, 0:NQ],
                                ALU.mult)
        nc.vector.tensor_copy(selc[D:P, :], combT[D:P, 0:NQ])
        rep_c2 = sing.tile([QV, NQ], BF16, name="rep_c2")
        nc.gpsimd.partition_broadcast(rep_c2[:], cxr[0:1, 0:NQ])
        qcl2 = sing.tile([QV, NQ], BF16, name="qcl2")
        nc.vector.tensor_tensor(qcl2[:], qvT[:], rep_c2[:], ALU.mult)
        build_z(selc)
        pc = psY.tile([P, NQ], F32, tag="Y", name="pc")
        hyper_mms(pc[0:D, :], D, wzc, smalls[:, 256:320], smalls[:, 320:384],
                  qcl2)
        cand = sing.tile([D, NQ], BF16, name="cand")
        nc.scalar.activation(cand[:], pc[0:D, :], ACTF.Tanh)

        # out = hn + u*(cand - hn)   (hn = selc rows 0:64, u = ru rows 64:128)
        u64 = sing.tile([D, NQ], BF16, name="u64")
        nc.vector.tensor_copy(u64[:], ru[D:P, :])
        d1 = sing.tile([D, NQ], BF16, name="d1")
        nc.vector.tensor_tensor(d1[:], cand[:], selc[0:D, :], ALU.subtract)
        nc.vector.tensor_tensor(d1[:], d1[:], u64[:], ALU.mult)
        outT = sing.tile([D, NQ], F32, name="outT")
        nc.vector.tensor_tensor(outT[:], d1[:], selc[0:D, :], ALU.add)
        for j in range(4):
            pt = psY.tile([P, D], F32, tag="Y", name="pt")
            nc.tensor.transpose(pt[:, 0:D], outT[:, ts(j, P)],
                                identf[0:D, 0:D])
            ob = work.tile([P, D], F32, tag="ob", name="ob")
            nc.vector.tensor_copy(ob[:], pt[:, 0:D])
            nc.sync.dma_start(out_d[ts(j, P), :], ob[:])

    return nc


_NC_CACHE = {}


def _get_nc():
    key = bool(USE_LRELU[0])
    if key not in _NC_CACHE:
        nc = build_graph(hw_leaky=key)
        if not nc.is_finalized():
            nc.finalize()
        _NC_CACHE[key] = nc
    return _NC_CACHE[key]


# channel reorder: new order = [h (64) | x (65)]
_R = np.concatenate([np.arange(65, 129), np.arange(0, 65)])
_BF = ml_dtypes.bfloat16


def _bf(a):
    return np.ascontiguousarray(np.asarray(a, np.float32).astype(_BF))


def _prep_shared(Wq, bq, Wk, bk, Wv, bv, W1, b1, W2, b2, Wr, br, Wu, bu, Wc, bc):
    f32 = np.float32
    Wq, bq = np.asarray(Wq, f32), np.asarray(bq, f32)
    Wk, bk = np.asarray(Wk, f32), np.asarray(bk, f32)
    Wv, bv = np.asarray(Wv, f32), np.asarray(bv, f32)
    W1, b1 = np.asarray(W1, f32).reshape(H, C, C), np.asarray(b1, f32)
    W2, b2 = np.asarray(W2, f32), np.asarray(b2, f32)

    # qk packed: per pair cols [k_h0|0|q_h0/4|0|k_h1|0|q_h1/4|0] (16 each);
    # contraction rows = 128 reordered channels + c128; biases ride the
    # psum->sbuf copies as per-partition bias APs (bkq).
    Wq_r, Wk_r = Wq[:, _R, :], Wk[:, _R, :]
    kqw = np.zeros((C, 256), f32)
    bkq = np.zeros((KD, 2 * H), f32)
    for h in range(H):
        base = (h // 2) * 128 + (h % 2) * 64
        kqw[0:129, base:base + 16] = Wk_r[h]
        kqw[0:129, base + 32:base + 48] = Wq_r[h] * 0.25
        bkq[:, h] = bk[h]
        bkq[:, H + h] = bq[h] * 0.25

    # V with W1 folded: U_h = Wv_h @ W1_h, contraction rows reordered
    U = np.stack([(Wv[h] @ W1[h])[_R] for h in range(H)])    # [H, 129, 129]
    wv1 = np.ascontiguousarray(
        np.transpose(U, (1, 0, 2)).reshape(C, H * C))        # [129, 516]
    b1_eff = b1 + sum(bv[h] @ W1[h] for h in range(H))

    w2r = W2[:, _R]                                          # cols reordered
    b2r = b2[_R]
    biases = np.zeros((C, 4), f32)
    biases[:, 0] = b1_eff
    biases[:, 1] = b2r

    Wr_r = np.asarray(Wr, f32)[:, _R, :]
    Wu_r = np.asarray(Wu, f32)[:, _R, :]
    Wc_r = np.asarray(Wc, f32)[:, _R, :]
    wzru = np.ascontiguousarray(np.transpose(
        np.concatenate([Wr_r[:, 0:128, :], Wu_r[:, 0:128, :]], 2),
        (1, 0, 2)).reshape(P, QV * P))
    wzc = np.ascontiguousarray(
        np.transpose(Wc_r[:, 0:128, :], (1, 0, 2)).reshape(P, QV * D))
    smalls = np.zeros((QV, 384), f32)
    smalls[:, 0:64] = Wr_r[:, 128, :]
    smalls[:, 64:128] = Wu_r[:, 128, :]
    smalls[:, 128:192] = np.asarray(br, f32)
    smalls[:, 192:256] = np.asarray(bu, f32)
    smalls[:, 256:320] = Wc_r[:, 128, :]
    smalls[:, 320:384] = np.asarray(bc, f32)

    return dict(
        kqw=_bf(kqw[0:128]), kqwL=_bf(kqw[128:129]),
        bkq=np.ascontiguousarray(bkq),
        wv1a=_bf(wv1[0:128]), wv1b=_bf(wv1[128:129]),
        w2a=_bf(w2r[0:128]), w2b=_bf(w2r[128:129]),
        biases=np.ascontiguousarray(biases[0:128]),
        biasesL=np.ascontiguousarray(biases[128:129]),
        wzru=_bf(wzru), wzc=_bf(wzc), smalls=_bf(smalls),
    )


def _prep_core(b, x, h, query_vectors, adj, nodes_n, shared):
    idx = nodes_n[b * NQ:(b + 1) * NQ].astype(np.int64)
    rest = np.setdiff1d(np.arange(N, dtype=np.int64), idx)
    perm = np.concatenate([idx, rest])
    comb = np.concatenate([x[b][perm], h[b][perm]], 1)[:, _R]  # [N,129] reord
    combT = np.ascontiguousarray(comb.T.astype(_BF))           # [129, N]
    qv = query_vectors[b * NQ:(b + 1) * NQ]                    # [512, 32]
    qvT = np.ascontiguousarray(qv.T.astype(_BF))               # [32, 512]
    # qb [128, 32*512]: qb[p, d*512 + m] = qv[m, d]  (uint16 view = fast)
    qvT_u16 = qvT.view(np.uint16)
    qb = np.ascontiguousarray(
        np.broadcast_to(qvT_u16[None, :, :], (P, QV, NQ))
        .reshape(P, QV * NQ)).view(_BF)
    adj01 = (adj[np.ix_(idx, perm)] != 0).astype(np.float32).T  # [N, 512]
    adjT = np.ascontiguousarray(
        adj01.reshape(NT, P, NQ).transpose(1, 0, 2).reshape(P, NT * NQ)
    ).astype(_BF)
    d = dict(shared)
    d["combT"] = np.ascontiguousarray(combT[0:128])
    d["c128"] = np.ascontiguousarray(combT[128:129])
    d["qvT"] = qvT
    d["qb"] = qb
    d["adjT"] = adjT
    return d


def make_in_maps(x, h, query_vectors, adj, nodes_b, nodes_n, **weights):
    x = np.asarray(x, np.float32)
    h = np.asarray(h, np.float32)
    query_vectors = np.asarray(query_vectors, np.float32)
    adj = np.asarray(adj)
    nodes_n = np.asarray(nodes_n)
    shared = _prep_shared(**weights)
    return [_prep_core(b, x, h, query_vectors, adj, nodes_n, shared)
            for b in range(B)]


def kernel(x, h, query_vectors, adj, nodes_b, nodes_n,
           Wq, bq, Wk, bk, Wv, bv, W1, b1, W2, b2,
           Wr, br, Wu, bu, Wc, bc):
    in_maps = make_in_maps(
        x, h, query_vectors, adj, nodes_b, nodes_n,
        Wq=Wq, bq=bq, Wk=Wk, bk=bk, Wv=Wv, bv=bv, W1=W1, b1=b1, W2=W2, b2=b2,
        Wr=Wr, br=br, Wu=Wu, bu=bu, Wc=Wc, bc=bc)
    nc = _get_nc()
    res = run_bass_kernel_spmd(nc, in_maps, list(range(B)))
    outs = [np.asarray(res.results[b]["out"], np.float32) for b in range(B)]
    return np.concatenate(outs, axis=0)
